# revision 37
# baseline (speedup 1.0000x reference)
"""Trainium2 Bass kernel for nn_GPTQOFTLinear.

y = (x rotated by block-diagonal Cayley(oft_r)) @ W^T + b

Strategy (8 NeuronCores, no collectives):
  - Data-parallel shard x over the 8192 tokens (1024 tokens/core); W, oft_r, b
    replicated.
  - On each core:
      1. Cayley transform packed as 32 block-diagonal 128x128 pairs:
         Q = F (I - C)^{-1} with S = skew(data), C = S@S, F = (I-S)^2
           = I - 2S + C.  The inverse is approximated by the 2-term Horner
         series (I - C)^{-1} ~= I + C + C^2 (|C|~0.05 so the truncation
         error ~|C|^3 ~ 1e-4).  3 matmuls/pair, fp16 operands, emitted in
         batched phases so the PE stream stays dense:
            psC = s2^T s2 = -4C          (s2 = 2S)
            psX = C @ (I + C)
            psQ = (C - 2S) @ X,   Q = X + psQ   (X = I + C + C^2)
      2. Rotate: x_rot^T[:, j] = Q_j^T-free matmuls (fp32r), result copied
         into a bf16 SBUF-resident x_rot^T [128, 32, 1024].
      3. Main matmul in bf16: y[t, o] = sum_j x_rot^T[j, t] * W^T[j, o] + b.
         W^T arrives per output-group as a 32 KiB/partition bf16 SBUF tile
         via a single gpsimd *casting* DMA (f32 HBM -> bf16 SBUF), so the
         inner loop has no DMA waits; 8 PSUM banks accumulate 8 token tiles.
  - Host side does only layout (shard/transpose/zero-pad/replicate), no math.
"""

import os
import sys

for _p in ("/opt/trn_rl_repo",):
    if _p not in sys.path and os.path.isdir(_p):
        sys.path.append(_p)

import numpy as np

import concourse.bass as bass  # noqa: E402
import concourse.mybir as mybir  # noqa: E402
import concourse.tile as tile  # noqa: E402
from concourse import bacc  # noqa: E402
from concourse.bass_utils import run_bass_kernel_spmd  # noqa: E402

# Problem shapes (hardcoded per contract).
BATCH, SEQ = 2, 4096
DIN = 4096
DOUT = 4096
BS = 64                      # oft block size
RANK = DIN // BS             # 64 blocks
N_CORES = 8
TOK = BATCH * SEQ            # 8192 tokens
TPC = TOK // N_CORES         # 1024 tokens per core
P = 128
JT = DIN // P                # 32 contraction tiles
NPAIR = RANK // 2            # 32 block pairs
NT = TPC // P                # 8 token tiles per core
OGW = 512                    # output-feature group width
OG = DOUT // OGW             # 8 output groups
XCH = 2                      # x^T j-tiles per staged DMA chunk
JH = JT // 2                 # j-tiles per W og-half tile
ALU = mybir.AluOpType

F32 = mybir.dt.float32
F32R = mybir.dt.float32r
F16 = mybir.dt.float16
BF16 = mybir.dt.bfloat16
FP8 = mybir.dt.float8e4
WSCALE = 64.0                # W pre-scale into e4m3 normal range

# fp8:  3-term hi/lo e4m3 split, DoubleRow matmuls (2 rows/cycle).
# bf16: W tiles og-resident in SBUF via casting DMA, xrot bf16.
# f32r: baseline-style streamed f32r W tiles, xrot f32r.
_MODE = os.environ.get("KERNEL_MODE", "bf16")

_CACHE: dict = {}


def _emit_cayley(nc, tc, g_all, gt_all, eye_sb, Q, post_phase1=None):
    """Q[:, p, :] = Cayley(pair p), batched phases, fp16 operands.

    g_all/gt_all are the densely packed [P, NPAIR, BS] f32 tiles: partition
    quadrant 0:64 holds block 2p, 64:128 holds block 2p+1."""
    from contextlib import ExitStack

    with ExitStack() as ctx:
        arr = ctx.enter_context(tc.tile_pool(name="cayarr", bufs=1))
        cps = ctx.enter_context(tc.tile_pool(name="cps", bufs=4, space="PSUM"))
        xps = ctx.enter_context(tc.tile_pool(name="xps", bufs=2, space="PSUM"))
        qps = ctx.enter_context(tc.tile_pool(name="qps", bufs=2, space="PSUM"))

        def veng(i):
            return nc.vector if i % 2 == 0 else nc.gpsimd

        # Pair-groups of 8: separate tiles per group keep the dependency
        # tracker (whole-tile granularity) from serializing phases — group g
        # computes while group g+1's inputs land.
        GRP = 8
        NG = NPAIR // GRP

        # NOTE: gpsimd (Pool) cannot access PSUM on TRN2; PSUM-reading ops go
        # on DVE (tensor_tensor) or Activation (copy/scale only).
        for g in range(NG):
            s2 = arr.tile([P, GRP, P], F16, name=f"s2{g}", tag=f"s2{g}")
            C = arr.tile([P, GRP, P], F16, name=f"C{g}", tag=f"C{g}")
            X = arr.tile([P, GRP, P], F16, name=f"X{g}", tag=f"X{g}")

            # phase 1: s2 = g - gt (= 2S), block-diagonal; off-diagonal
            # quadrants zeroed (overlaps the g/gt DMA for later groups).
            nc.vector.memset(s2[:BS, :, BS:], 0)
            nc.gpsimd.memset(s2[BS:, :, :BS], 0)
            for i in range(GRP):
                p = g * GRP + i
                veng(p).tensor_sub(
                    s2[:BS, i, :BS], g_all[:BS, p, :], gt_all[:BS, p, :])
                veng(p + 1).tensor_sub(
                    s2[BS:, i, BS:], g_all[BS:, p, :], gt_all[BS:, p, :])
            if g == 0 and post_phase1 is not None:
                post_phase1()

            # phase 2: psC = s2^T @ s2 = -4C ; C = -0.25 psC (Act scale-copy)
            pcs = []
            for i in range(GRP):
                ps = cps.tile([P, P], F32, name="cps", tag="cps")
                nc.tensor.matmul(ps, s2[:, i, :], s2[:, i, :])
                pcs.append(ps)
            for i in range(GRP):
                nc.scalar.mul(C[:, i, :], pcs[i], -0.25)

            # phase 3: psX = C^T @ C = C^2 ; X = psX + C (DVE), X += I (gps)
            pxs = []
            for i in range(GRP):
                ps = xps.tile([P, P], F32, name="xps", tag="xps")
                nc.tensor.matmul(ps, C[:, i, :], C[:, i, :])
                pxs.append(ps)
            for i in range(GRP):
                nc.vector.tensor_add(X[:, i, :], pxs[i], C[:, i, :])
                nc.gpsimd.tensor_add(X[:, i, :], X[:, i, :], eye_sb)

            # phase 4: psQ = (C - 2S) @ X via two accumulating matmuls;
            #          Q = X + psQ = F @ X with F = I - 2S + C.
            for i in range(GRP):
                p = g * GRP + i
                ps = qps.tile([P, P], F32, name="qps", tag="qps")
                nc.tensor.matmul(ps, C[:, i, :], X[:, i, :],
                                 start=True, stop=False)
                nc.tensor.matmul(ps, s2[:, i, :], X[:, i, :],
                                 start=False, stop=True)
                nc.vector.tensor_add(Q[g][:, i, :], ps, X[:, i, :])


def _emit(nc, tc, xTr, wTr, G, Gt, eye, bias_rep, y):
    """Emit the whole per-core program under TileContext tc."""
    from contextlib import ExitStack

    xrot_dt = BF16 if _MODE == "bf16" else F32R

    ctx = ExitStack()
    with ctx:
        # ---- persistent pools (allocated first, stable addresses) ----
        const = ctx.enter_context(tc.tile_pool(name="const", bufs=1))
        qpool = ctx.enter_context(tc.tile_pool(name="qpool", bufs=1))
        xrot_pool = ctx.enter_context(tc.tile_pool(name="xrotp", bufs=1))
        if _MODE == "bf16":
            wt_pool = ctx.enter_context(tc.tile_pool(name="wtp", bufs=3))
        elif _MODE == "fp8":
            wt_pool = ctx.enter_context(tc.tile_pool(name="wtp", bufs=3))
            wh_pool = ctx.enter_context(tc.tile_pool(name="whp", bufs=3))
            wl_pool = ctx.enter_context(tc.tile_pool(name="wlp", bufs=3))
        else:
            wt_pool = ctx.enter_context(tc.tile_pool(name="wtp", bufs=6))
        out_pool = ctx.enter_context(tc.tile_pool(name="outp", bufs=4))
        bias_pool = ctx.enter_context(tc.tile_pool(name="biasp", bufs=1))
        xstage_pool = ctx.enter_context(
            tc.tile_pool(name="xstagep", bufs=2 if _MODE == "fp8" else 3))

        cayio_pool = ctx.enter_context(tc.tile_pool(name="cayio", bufs=1))

        # DMA priority order on the sync queue: eye + Cayley inputs first
        # (tiny, gate the whole pipeline), then x chunks, then per-og bias.
        eye_sb = const.tile([P, P], F32, name="eye_sb", tag="eye")
        nc.sync.dma_start(out=eye_sb, in_=eye)
        g_all = cayio_pool.tile([P, NPAIR, BS], F32, name="g_all", tag="g_all")
        nc.sync.dma_start(out=g_all, in_=G)
        gt_all = cayio_pool.tile([P, NPAIR, BS], F32, name="gt_all",
                                 tag="gt_all")
        nc.sync.dma_start(out=gt_all, in_=Gt)

        Q = [qpool.tile([P, 8, P], F32R, name=f"Q{g}", tag=f"Q{g}")
             for g in range(NPAIR // 8)]
        if _MODE == "fp8":
            xh = xrot_pool.tile([P, JT, TPC], FP8, name="xh", tag="xh")
            xl = xrot_pool.tile([P, JT, TPC], FP8, name="xl", tag="xl")
            xrot = None
        else:
            xrot = xrot_pool.tile([P, JT, TPC], xrot_dt, name="xrot",
                                  tag="xrot")

        # x^T staged chunks, spread over THREE DMA initiator queues (sync,
        # Act HWDGE, gpsimd SWDGE) — transfers on one queue serialize, so a
        # single queue caps x at ~1/2 bandwidth.  Chunks 0-9 alternate
        # sync/Act up front; chunks 10+ go on gpsimd after the Cayley
        # phase-1 vector work (so their desc-gen doesn't delay it).
        NCH = JT // XCH
        xs_tiles = [
            xstage_pool.tile([P, XCH, TPC], F32R, name="xs", tag="xs")
            for _ in range(NCH)
        ]

        def issue_xs(c, eng):
            eng.dma_start(out=xs_tiles[c], in_=xTr[:, c * XCH:(c + 1) * XCH, :])

        for c in range(NCH - 2):
            issue_xs(c, nc.sync if c % 2 == 0 else nc.scalar)

        def gpsimd_xs():
            for c in range(NCH - 2, NCH):
                issue_xs(c, nc.gpsimd)

        # W og-half-tile prefetch (bf16 mode): gpsimd casting DMA f32 -> bf16
        # (casting DMAs are SWDGE-only).  Halves (16 j-tiles each) give finer
        # prefetch granularity at 3 bufs; the gpsimd queue carries only 2 x
        # chunks so og0's halves land before the main loop starts.
        wt_tiles = {}

        def issue_wt(og, half):
            if _MODE != "bf16" or og >= OG:
                return
            wt = wt_pool.tile([P, JH, OGW], BF16, name="wt", tag="wt")
            nc.gpsimd.dma_start(
                out=wt,
                in_=wTr[:, half * JH:(half + 1) * JH,
                        og * OGW:(og + 1) * OGW])
            wt_tiles[(og, half)] = wt

        # fp8 mode: W streams in as f32 j-pair tiles (sync/Act queues) and is
        # quantized on device into og-resident hi/lo e4m3 half-tiles:
        #   wh = e4m3(64 W),  wl = e4m3(64 W - wh)
        whl_tiles = {}

        def issue_wq(og):
            if _MODE != "fp8" or og >= OG:
                return
            wh = [wh_pool.tile([P, JH, OGW], FP8, name="wh", tag="wh")
                  for _ in range(2)]
            wl = [wl_pool.tile([P, JH, OGW], FP8, name="wl", tag="wl")
                  for _ in range(2)]
            for jp in range(JT // 2):
                wt = wt_pool.tile([P, 2, OGW], F32, name="wt", tag="wt")
                eng = nc.sync if jp % 2 == 0 else nc.scalar
                eng.dma_start(
                    out=wt,
                    in_=wTr[:, 2 * jp:2 * jp + 2, og * OGW:(og + 1) * OGW])
                h, i = jp // 8, jp % 8
                nc.scalar.mul(wh[h][:, 2 * i:2 * i + 2, :], wt[:], WSCALE)
                nc.vector.scalar_tensor_tensor(
                    wl[h][:, 2 * i:2 * i + 2, :], wt[:], WSCALE,
                    wh[h][:, 2 * i:2 * i + 2, :], ALU.mult, ALU.subtract)
            whl_tiles[og] = (wh, wl)

        # ---- Cayley (scoped; PSUM/SBUF freed before rotation/main) ----
        _emit_cayley(nc, tc, g_all, gt_all, eye_sb, Q, post_phase1=gpsimd_xs)

        # ---- rotation: x_rot^T[:, j, :] = Q_j^T.T @ x^T[j-tile] ----
        # (gpsimd cannot read PSUM: copies alternate Act / DVE)
        # W og0 prefetch starts late in the rotation so it doesn't steal DMA
        # bandwidth from the latency-critical x chunks.
        with tc.tile_pool(name="rpsum", bufs=4, space="PSUM") as rpsum:
            for j in range(JT):
                if j == 20:
                    issue_wt(0, 0)
                elif j == 30:
                    issue_wt(0, 1)
                xs = xs_tiles[j // XCH]
                for th in range(TPC // 512):
                    rps = rpsum.tile([P, 512], F32, name="rps", tag="rps")
                    nc.tensor.matmul(
                        rps,
                        Q[j // 8][:, j % 8, :],
                        xs[:, j % XCH, th * 512:(th + 1) * 512],
                    )
                    sl = slice(th * 512, (th + 1) * 512)
                    if _MODE == "fp8":
                        # xh = e4m3(x_rot); xl = e4m3(x_rot - xh)
                        nc.scalar.copy(out=xh[:, j, sl], in_=rps)
                        nc.vector.scalar_tensor_tensor(
                            xl[:, j, sl], rps, 1.0, xh[:, j, sl],
                            ALU.mult, ALU.subtract)
                    elif (2 * j + th) % 2 == 0:
                        nc.scalar.copy(out=xrot[:, j, sl], in_=rps)
                    else:
                        nc.vector.tensor_copy(out=xrot[:, j, sl], in_=rps)

        # ---- main matmul (all 8 PSUM banks) ----
        issue_wq(0)
        with tc.tile_pool(name="mpsum", bufs=1, space="PSUM") as mpsum:
            for og in range(OG):
                issue_wq(og + 1)
                bias_og = bias_pool.tile([P, OGW], F32, name="bias_og",
                                         tag="bias_og")
                nc.sync.dma_start(
                    out=bias_og, in_=bias_rep[:, og * OGW:(og + 1) * OGW])

                # Two 4-bank passes per og (token tiles 0-3, then 4-7), on
                # alternating PSUM bank groups: pass k+1 never waits on pass
                # k's drains, so og boundaries cost nothing.
                for ps in range(2):
                    grp = "A" if ps == 0 else "B"
                    psums = [
                        mpsum.tile([P, OGW], F32, name=f"mp{grp}{t4}",
                                   tag=f"mp{grp}{t4}")
                        for t4 in range(4)
                    ]
                    if _MODE == "fp8":
                        wh, wl = whl_tiles[og]
                        NJP = JT // 2
                        for jp in range(NJP):
                            h, i = jp // 8, jp % 8
                            jsl = slice(2 * i, 2 * i + 2)
                            for t4 in range(4):
                                tt = ps * 4 + t4
                                tsl = slice(tt * P, (tt + 1) * P)
                                for lhs, rhs, ti in (
                                        (xh, wh[h], 0),
                                        (xl, wh[h], 1),
                                        (xh, wl[h], 2)):
                                    nc.tensor.matmul(
                                        psums[t4],
                                        lhs[:, 2 * jp:2 * jp + 2, tsl],
                                        rhs[:, jsl, :],
                                        start=(jp == 0 and ti == 0),
                                        stop=(jp == NJP - 1 and ti == 2),
                                        perf_mode=(
                                            mybir.MatmulPerfMode.DoubleRow),
                                    )
                    elif _MODE == "bf16":
                        for half in range(2):
                            if ps == 0:
                                issue_wt(og + 1, half)
                            wt = wt_tiles[(og, half)]
                            for jj in range(JH):
                                j = half * JH + jj
                                for t4 in range(4):
                                    tt = ps * 4 + t4
                                    nc.tensor.matmul(
                                        psums[t4],
                                        xrot[:, j, tt * P:(tt + 1) * P],
                                        wt[:, jj, :],
                                        start=(j == 0),
                                        stop=(j == JT - 1),
                                    )
                    else:
                        for j in range(JT):
                            wt = wt_pool.tile([P, OGW], F32R, name="wt",
                                              tag="wt")
                            nc.sync.dma_start(
                                out=wt,
                                in_=wTr[:, j, og * OGW:(og + 1) * OGW])
                            for t4 in range(4):
                                tt = ps * 4 + t4
                                nc.tensor.matmul(
                                    psums[t4],
                                    xrot[:, j, tt * P:(tt + 1) * P],
                                    wt[:],
                                    start=(j == 0),
                                    stop=(j == JT - 1),
                                )
                    # Drains: DVE adds bias directly from PSUM; odd tiles go
                    # Act copy (PSUM->SBUF) + gpsimd bias add (SBUF only).
                    for t4 in range(4):
                        tt = ps * 4 + t4
                        out_sb = out_pool.tile([P, OGW], F32, name="out_sb",
                                               tag="out_sb")
                        if _MODE == "fp8":
                            # psum holds 64*(y - b): scale down + bias.
                            if t4 % 2 == 0:
                                nc.vector.scalar_tensor_tensor(
                                    out_sb, psums[t4], 1.0 / WSCALE, bias_og,
                                    ALU.mult, ALU.add)
                            else:
                                nc.scalar.mul(out_sb, psums[t4], 1.0 / WSCALE)
                                nc.gpsimd.tensor_add(out_sb, out_sb, bias_og)
                        elif t4 % 2 == 0:
                            nc.vector.tensor_add(out_sb, psums[t4], bias_og)
                        else:
                            nc.scalar.copy(out=out_sb, in_=psums[t4])
                            nc.gpsimd.tensor_add(out_sb, out_sb, bias_og)
                        yeng = nc.sync if t4 % 2 == 0 else nc.scalar
                        yeng.dma_start(
                            out=y[tt * P:(tt + 1) * P,
                                  og * OGW:(og + 1) * OGW],
                            in_=out_sb)


def _build():
    key = _MODE
    if key in _CACHE:
        return _CACHE[key]
    nc = bacc.Bacc("TRN2", target_bir_lowering=False, debug=False,
                   num_devices=N_CORES)
    xTr = nc.dram_tensor("xTr", [P, JT, TPC], F32R, kind="ExternalInput").ap()
    wTr = nc.dram_tensor("wTr", [P, JT, DOUT], F32, kind="ExternalInput").ap()
    G = nc.dram_tensor("G", [P, NPAIR, BS], F32, kind="ExternalInput").ap()
    Gt = nc.dram_tensor("Gt", [P, NPAIR, BS], F32, kind="ExternalInput").ap()
    eye = nc.dram_tensor("eye", [P, P], F32, kind="ExternalInput").ap()
    bias_rep = nc.dram_tensor("bias_rep", [P, DOUT], F32,
                              kind="ExternalInput").ap()
    y = nc.dram_tensor("y", [TPC, DOUT], F32, kind="ExternalOutput").ap()

    with tile.TileContext(nc) as tc:
        _emit(nc, tc, xTr, wTr, G, Gt, eye, bias_rep, y)
    nc.compile()
    _CACHE[key] = nc
    return nc


def _maybe_enable_trace():
    """Inject the NTFF profile hook so run_bass_kernel_spmd(trace=True) works
    under axon in this container.  Only used by the dev harness."""
    import types
    try:
        import antenv
        from trn_agent_boot.trn_boot import _ntff_profile_via_ctypes
        import concourse.bass_utils as bass_utils
        hook = _ntff_profile_via_ctypes("/opt/axon/libaxon_pjrt.so")
        mod = types.ModuleType("antenv.axon_hooks")
        mod.get_axon_ntff_profile_hook = lambda: hook
        mod.set_axon_ntff_profile_hook = lambda h: None
        sys.modules["antenv.axon_hooks"] = mod
        antenv.axon_hooks = mod
        bass_utils.upload_artifacts = lambda tmpdir: "local://" + tmpdir
        return True
    except Exception:
        return False


LAST_RESULT = None


def kernel(x, oft_r, W, b):
    global LAST_RESULT
    x = np.ascontiguousarray(np.asarray(x, dtype=np.float32))
    oft_r = np.asarray(oft_r, dtype=np.float32)
    W = np.asarray(W, dtype=np.float32)
    b = np.asarray(b, dtype=np.float32)

    nc = _build()

    # Host-side layout only (no arithmetic): shard/transpose/pad/replicate.
    xf = x.reshape(TOK, DIN)
    wTr = np.ascontiguousarray(
        W.T.reshape(JT, P, DOUT).transpose(1, 0, 2))
    # Dense block packing: partitions 0:64 hold block 2p, 64:128 block 2p+1.
    G = np.zeros((P, NPAIR, BS), np.float32)
    Gt = np.zeros((P, NPAIR, BS), np.float32)
    oft_t = oft_r.transpose(0, 2, 1)
    for p in range(NPAIR):
        G[:BS, p, :] = oft_r[2 * p]
        G[BS:, p, :] = oft_r[2 * p + 1]
        Gt[:BS, p, :] = oft_t[2 * p]
        Gt[BS:, p, :] = oft_t[2 * p + 1]
    eye = np.eye(P, dtype=np.float32)
    bias_rep = np.ascontiguousarray(np.broadcast_to(b, (P, DOUT)))

    shared = {"wTr": wTr, "G": G, "Gt": Gt, "eye": eye, "bias_rep": bias_rep}
    in_maps = []
    for c in range(N_CORES):
        xTc = np.ascontiguousarray(
            xf[c * TPC:(c + 1) * TPC].T.reshape(JT, P, TPC).transpose(1, 0, 2))
        in_maps.append({"xTr": xTc, **shared})

    trace = os.environ.get("KERNEL_TRACE", "0") == "1" and _maybe_enable_trace()
    res = run_bass_kernel_spmd(
        nc, in_maps, core_ids=list(range(N_CORES)), trace=trace,
        trace_cores=[0] if trace else None,
    )
    LAST_RESULT = res

    y = np.concatenate([res.results[c]["y"] for c in range(N_CORES)], axis=0)
    return np.ascontiguousarray(y.reshape(BATCH, SEQ, DOUT))


# revision 38
# speedup vs baseline: 1.4054x; 1.4054x over previous
"""Trainium2 Bass kernel for nn_GPTQOFTLinear.

y = (x rotated by block-diagonal Cayley(oft_r)) @ W^T + b

Strategy (8 NeuronCores, no collectives):
  - Data-parallel shard x over the 8192 tokens (1024 tokens/core); W, oft_r, b
    replicated.
  - On each core:
      1. Cayley transform packed as 32 block-diagonal 128x128 pairs:
         Q = F (I - C)^{-1} with S = skew(data), C = S@S, F = (I-S)^2
           = I - 2S + C.  The inverse is approximated by the 2-term Horner
         series (I - C)^{-1} ~= I + C + C^2 (|C|~0.05 so the truncation
         error ~|C|^3 ~ 1e-4).  3 matmuls/pair, fp16 operands, emitted in
         batched phases so the PE stream stays dense:
            psC = s2^T s2 = -4C          (s2 = 2S)
            psX = C @ (I + C)
            psQ = (C - 2S) @ X,   Q = X + psQ   (X = I + C + C^2)
      2. Rotate: x_rot^T[:, j] = Q_j^T-free matmuls (fp32r), result copied
         into a bf16 SBUF-resident x_rot^T [128, 32, 1024].
      3. Main matmul in bf16: y[t, o] = sum_j x_rot^T[j, t] * W^T[j, o] + b.
         W^T arrives per output-group as a 32 KiB/partition bf16 SBUF tile
         via a single gpsimd *casting* DMA (f32 HBM -> bf16 SBUF), so the
         inner loop has no DMA waits; 8 PSUM banks accumulate 8 token tiles.
  - Host side does only layout (shard/transpose/zero-pad/replicate), no math.
"""

import os
import sys

for _p in ("/opt/trn_rl_repo",):
    if _p not in sys.path and os.path.isdir(_p):
        sys.path.append(_p)

import numpy as np

import concourse.bass as bass  # noqa: E402
import concourse.mybir as mybir  # noqa: E402
import concourse.tile as tile  # noqa: E402
from concourse import bacc  # noqa: E402
from concourse.bass_utils import run_bass_kernel_spmd  # noqa: E402

# Problem shapes (hardcoded per contract).
BATCH, SEQ = 2, 4096
DIN = 4096
DOUT = 4096
BS = 64                      # oft block size
RANK = DIN // BS             # 64 blocks
N_CORES = 8
TOK = BATCH * SEQ            # 8192 tokens
TPC = TOK // N_CORES         # 1024 tokens per core
P = 128
JT = DIN // P                # 32 contraction tiles
NPAIR = RANK // 2            # 32 block pairs
NT = TPC // P                # 8 token tiles per core
OGW = 512                    # output-feature group width
OG = DOUT // OGW             # 8 output groups
XCH = 2                      # x^T j-tiles per staged DMA chunk
JH = JT // 2                 # j-tiles per W og-half tile
ALU = mybir.AluOpType

F32 = mybir.dt.float32
F32R = mybir.dt.float32r
F16 = mybir.dt.float16
BF16 = mybir.dt.bfloat16

# bf16: W tiles og-resident in SBUF via casting DMA, xrot bf16.
# f32r: baseline-style streamed f32r W tiles, xrot f32r.
_MODE = os.environ.get("KERNEL_MODE", "bf16")

_CACHE: dict = {}


def _emit_cayley(nc, tc, g_all, gt_all, eye_sb, Q, post_phase1=None):
    """Q[:, p, :] = Cayley(pair p), batched phases, fp16 operands.

    g_all/gt_all are the densely packed [P, NPAIR, BS] f32 tiles: partition
    quadrant 0:64 holds block 2p, 64:128 holds block 2p+1."""
    from contextlib import ExitStack

    with ExitStack() as ctx:
        arr = ctx.enter_context(tc.tile_pool(name="cayarr", bufs=1))
        cps = ctx.enter_context(tc.tile_pool(name="cps", bufs=4, space="PSUM"))
        xps = ctx.enter_context(tc.tile_pool(name="xps", bufs=2, space="PSUM"))
        qps = ctx.enter_context(tc.tile_pool(name="qps", bufs=2, space="PSUM"))

        def veng(i):
            return nc.vector if i % 2 == 0 else nc.gpsimd

        # Pair-groups of 8: separate tiles per group keep the dependency
        # tracker (whole-tile granularity) from serializing phases — group g
        # computes while group g+1's inputs land.
        GRP = 8
        NG = NPAIR // GRP

        # NOTE: gpsimd (Pool) cannot access PSUM on TRN2; PSUM-reading ops go
        # on DVE (tensor_tensor) or Activation (copy/scale only).
        for g in range(NG):
            s2 = arr.tile([P, GRP, P], F16, name=f"s2{g}", tag=f"s2{g}")
            C = arr.tile([P, GRP, P], F16, name=f"C{g}", tag=f"C{g}")
            X = arr.tile([P, GRP, P], F16, name=f"X{g}", tag=f"X{g}")

            # phase 1: s2 = g - gt (= 2S), block-diagonal; off-diagonal
            # quadrants zeroed (overlaps the g/gt DMA for later groups).
            nc.vector.memset(s2[:BS, :, BS:], 0)
            nc.gpsimd.memset(s2[BS:, :, :BS], 0)
            for i in range(GRP):
                p = g * GRP + i
                veng(p).tensor_sub(
                    s2[:BS, i, :BS], g_all[:BS, p, :], gt_all[:BS, p, :])
                veng(p + 1).tensor_sub(
                    s2[BS:, i, BS:], g_all[BS:, p, :], gt_all[BS:, p, :])
            if g == 0 and post_phase1 is not None:
                post_phase1()

            # phase 2: psC = s2^T @ s2 = -4C ; C = -0.25 psC (Act scale-copy)
            pcs = []
            for i in range(GRP):
                ps = cps.tile([P, P], F32, name="cps", tag="cps")
                nc.tensor.matmul(ps, s2[:, i, :], s2[:, i, :])
                pcs.append(ps)
            for i in range(GRP):
                nc.scalar.mul(C[:, i, :], pcs[i], -0.25)

            # phase 3: psX = C^T @ C = C^2 ; X = psX + C (DVE), X += I (gps)
            pxs = []
            for i in range(GRP):
                ps = xps.tile([P, P], F32, name="xps", tag="xps")
                nc.tensor.matmul(ps, C[:, i, :], C[:, i, :])
                pxs.append(ps)
            for i in range(GRP):
                nc.vector.tensor_add(X[:, i, :], pxs[i], C[:, i, :])
                nc.gpsimd.tensor_add(X[:, i, :], X[:, i, :], eye_sb)

            # phase 4: psQ = (C - 2S) @ X via two accumulating matmuls;
            #          Q = X + psQ = F @ X with F = I - 2S + C.
            for i in range(GRP):
                p = g * GRP + i
                ps = qps.tile([P, P], F32, name="qps", tag="qps")
                nc.tensor.matmul(ps, C[:, i, :], X[:, i, :],
                                 start=True, stop=False)
                nc.tensor.matmul(ps, s2[:, i, :], X[:, i, :],
                                 start=False, stop=True)
                nc.vector.tensor_add(Q[g][:, i, :], ps, X[:, i, :])


def _emit(nc, tc, xTr, wTr, G, Gt, eye, bias_rep, y):
    """Emit the whole per-core program under TileContext tc."""
    from contextlib import ExitStack

    xrot_dt = BF16 if _MODE == "bf16" else F32R

    ctx = ExitStack()
    with ctx:
        # ---- persistent pools (allocated first, stable addresses) ----
        const = ctx.enter_context(tc.tile_pool(name="const", bufs=1))
        qpool = ctx.enter_context(tc.tile_pool(name="qpool", bufs=1))
        xrot_pool = ctx.enter_context(tc.tile_pool(name="xrotp", bufs=1))
        if _MODE == "bf16":
            wt_pool = ctx.enter_context(tc.tile_pool(name="wtp", bufs=3))
        else:
            wt_pool = ctx.enter_context(tc.tile_pool(name="wtp", bufs=6))
        out_pool = ctx.enter_context(tc.tile_pool(name="outp", bufs=4))
        bias_pool = ctx.enter_context(tc.tile_pool(name="biasp", bufs=2))
        xstage_pool = ctx.enter_context(tc.tile_pool(name="xstagep", bufs=3))

        cayio_pool = ctx.enter_context(tc.tile_pool(name="cayio", bufs=1))

        # DMA priority order on the sync queue: eye + Cayley inputs first
        # (tiny, gate the whole pipeline), then x chunks, then per-og bias.
        eye_sb = const.tile([P, P], F32, name="eye_sb", tag="eye")
        nc.sync.dma_start(out=eye_sb, in_=eye)
        g_all = cayio_pool.tile([P, NPAIR, BS], F32, name="g_all", tag="g_all")
        nc.sync.dma_start(out=g_all, in_=G)
        gt_all = cayio_pool.tile([P, NPAIR, BS], F32, name="gt_all",
                                 tag="gt_all")
        nc.sync.dma_start(out=gt_all, in_=Gt)

        Q = [qpool.tile([P, 8, P], F32R, name=f"Q{g}", tag=f"Q{g}")
             for g in range(NPAIR // 8)]
        xrot = xrot_pool.tile([P, JT, TPC], xrot_dt, name="xrot", tag="xrot")

        # x^T staged chunks, spread over THREE DMA initiator queues (sync,
        # Act HWDGE, gpsimd SWDGE) — transfers on one queue serialize, so a
        # single queue caps x at ~1/2 bandwidth.  Chunks 0-9 alternate
        # sync/Act up front; chunks 10+ go on gpsimd after the Cayley
        # phase-1 vector work (so their desc-gen doesn't delay it).
        NCH = JT // XCH
        xs_tiles = [
            xstage_pool.tile([P, XCH, TPC], F32R, name="xs", tag="xs")
            for _ in range(NCH)
        ]

        def issue_xs(c, eng):
            eng.dma_start(out=xs_tiles[c], in_=xTr[:, c * XCH:(c + 1) * XCH, :])

        for c in range(NCH - 2):
            issue_xs(c, nc.sync if c % 2 == 0 else nc.scalar)

        def gpsimd_xs():
            for c in range(NCH - 2, NCH):
                issue_xs(c, nc.gpsimd)

        # W og-half-tile prefetch (bf16 mode): gpsimd casting DMA f32 -> bf16
        # (casting DMAs are SWDGE-only).  Halves (16 j-tiles each) give finer
        # prefetch granularity at 3 bufs; the gpsimd queue carries only 2 x
        # chunks so og0's halves land before the main loop starts.
        wt_tiles = {}

        def issue_wt(og, half):
            if _MODE != "bf16" or og >= OG:
                return
            wt = wt_pool.tile([P, JH, OGW], BF16, name="wt", tag="wt")
            nc.gpsimd.dma_start(
                out=wt,
                in_=wTr[:, half * JH:(half + 1) * JH,
                        og * OGW:(og + 1) * OGW])
            wt_tiles[(og, half)] = wt

        # ---- Cayley (scoped; PSUM/SBUF freed before rotation/main) ----
        _emit_cayley(nc, tc, g_all, gt_all, eye_sb, Q, post_phase1=gpsimd_xs)

        # ---- rotation: x_rot^T[:, j, :] = Q_j^T.T @ x^T[j-tile] ----
        # (gpsimd cannot read PSUM: copies alternate Act / DVE)
        # W og0 prefetch starts late in the rotation so it doesn't steal DMA
        # bandwidth from the latency-critical x chunks.
        with tc.tile_pool(name="rpsum", bufs=4, space="PSUM") as rpsum:
            for j in range(JT):
                if j == 20:
                    issue_wt(0, 0)
                elif j == 30:
                    issue_wt(0, 1)
                xs = xs_tiles[j // XCH]
                for th in range(TPC // 512):
                    rps = rpsum.tile([P, 512], F32, name="rps", tag="rps")
                    nc.tensor.matmul(
                        rps,
                        Q[j // 8][:, j % 8, :],
                        xs[:, j % XCH, th * 512:(th + 1) * 512],
                    )
                    dst = xrot[:, j, th * 512:(th + 1) * 512]
                    if (2 * j + th) % 2 == 0:
                        nc.scalar.copy(out=dst, in_=rps)
                    else:
                        nc.vector.tensor_copy(out=dst, in_=rps)

        # ---- main matmul (all 8 PSUM banks) ----
        with tc.tile_pool(name="mpsum", bufs=1, space="PSUM") as mpsum:
            for og in range(OG):
                bias_og = bias_pool.tile([P, OGW], F32, name="bias_og",
                                         tag="bias_og")
                nc.sync.dma_start(
                    out=bias_og, in_=bias_rep[:, og * OGW:(og + 1) * OGW])

                # Two 4-bank passes per og (token tiles 0-3, then 4-7), on
                # alternating PSUM bank groups: pass k+1 never waits on pass
                # k's drains, so og boundaries cost nothing.
                for ps in range(2):
                    grp = "A" if ps == 0 else "B"
                    psums = [
                        mpsum.tile([P, OGW], F32, name=f"mp{grp}{t4}",
                                   tag=f"mp{grp}{t4}")
                        for t4 in range(4)
                    ]
                    if _MODE == "bf16":
                        for half in range(2):
                            if ps == 0:
                                issue_wt(og + 1, half)
                            wt = wt_tiles[(og, half)]
                            for jj in range(JH):
                                j = half * JH + jj
                                for t4 in range(4):
                                    tt = ps * 4 + t4
                                    nc.tensor.matmul(
                                        psums[t4],
                                        xrot[:, j, tt * P:(tt + 1) * P],
                                        wt[:, jj, :],
                                        start=(j == 0),
                                        stop=(j == JT - 1),
                                    )
                    else:
                        for j in range(JT):
                            wt = wt_pool.tile([P, OGW], F32R, name="wt",
                                              tag="wt")
                            nc.sync.dma_start(
                                out=wt,
                                in_=wTr[:, j, og * OGW:(og + 1) * OGW])
                            for t4 in range(4):
                                tt = ps * 4 + t4
                                nc.tensor.matmul(
                                    psums[t4],
                                    xrot[:, j, tt * P:(tt + 1) * P],
                                    wt[:],
                                    start=(j == 0),
                                    stop=(j == JT - 1),
                                )
                    # Drains: DVE adds bias directly from PSUM; odd tiles go
                    # Act copy (PSUM->SBUF) + gpsimd bias add (SBUF only).
                    for t4 in range(4):
                        tt = ps * 4 + t4
                        out_sb = out_pool.tile([P, OGW], F32, name="out_sb",
                                               tag="out_sb")
                        if t4 % 2 == 0:
                            nc.vector.tensor_add(out_sb, psums[t4], bias_og)
                        else:
                            nc.scalar.copy(out=out_sb, in_=psums[t4])
                            nc.gpsimd.tensor_add(out_sb, out_sb, bias_og)
                        yeng = nc.sync if t4 % 2 == 0 else nc.scalar
                        yeng.dma_start(
                            out=y[tt * P:(tt + 1) * P,
                                  og * OGW:(og + 1) * OGW],
                            in_=out_sb)


def _build():
    key = _MODE
    if key in _CACHE:
        return _CACHE[key]
    nc = bacc.Bacc("TRN2", target_bir_lowering=False, debug=False,
                   num_devices=N_CORES)
    xTr = nc.dram_tensor("xTr", [P, JT, TPC], F32R, kind="ExternalInput").ap()
    wTr = nc.dram_tensor("wTr", [P, JT, DOUT], F32, kind="ExternalInput").ap()
    G = nc.dram_tensor("G", [P, NPAIR, BS], F32, kind="ExternalInput").ap()
    Gt = nc.dram_tensor("Gt", [P, NPAIR, BS], F32, kind="ExternalInput").ap()
    eye = nc.dram_tensor("eye", [P, P], F32, kind="ExternalInput").ap()
    bias_rep = nc.dram_tensor("bias_rep", [P, DOUT], F32,
                              kind="ExternalInput").ap()
    y = nc.dram_tensor("y", [TPC, DOUT], F32, kind="ExternalOutput").ap()

    with tile.TileContext(nc) as tc:
        _emit(nc, tc, xTr, wTr, G, Gt, eye, bias_rep, y)
    nc.compile()
    _CACHE[key] = nc
    return nc


def _maybe_enable_trace():
    """Inject the NTFF profile hook so run_bass_kernel_spmd(trace=True) works
    under axon in this container.  Only used by the dev harness."""
    import types
    try:
        import antenv
        from trn_agent_boot.trn_boot import _ntff_profile_via_ctypes
        import concourse.bass_utils as bass_utils
        hook = _ntff_profile_via_ctypes("/opt/axon/libaxon_pjrt.so")
        mod = types.ModuleType("antenv.axon_hooks")
        mod.get_axon_ntff_profile_hook = lambda: hook
        mod.set_axon_ntff_profile_hook = lambda h: None
        sys.modules["antenv.axon_hooks"] = mod
        antenv.axon_hooks = mod
        bass_utils.upload_artifacts = lambda tmpdir: "local://" + tmpdir
        return True
    except Exception:
        return False


LAST_RESULT = None


def kernel(x, oft_r, W, b):
    global LAST_RESULT
    x = np.ascontiguousarray(np.asarray(x, dtype=np.float32))
    oft_r = np.asarray(oft_r, dtype=np.float32)
    W = np.asarray(W, dtype=np.float32)
    b = np.asarray(b, dtype=np.float32)

    nc = _build()

    # Host-side layout only (no arithmetic): shard/transpose/pad/replicate.
    xf = x.reshape(TOK, DIN)
    wTr = np.ascontiguousarray(
        W.T.reshape(JT, P, DOUT).transpose(1, 0, 2))
    # Dense block packing: partitions 0:64 hold block 2p, 64:128 block 2p+1.
    G = np.zeros((P, NPAIR, BS), np.float32)
    Gt = np.zeros((P, NPAIR, BS), np.float32)
    oft_t = oft_r.transpose(0, 2, 1)
    for p in range(NPAIR):
        G[:BS, p, :] = oft_r[2 * p]
        G[BS:, p, :] = oft_r[2 * p + 1]
        Gt[:BS, p, :] = oft_t[2 * p]
        Gt[BS:, p, :] = oft_t[2 * p + 1]
    eye = np.eye(P, dtype=np.float32)
    bias_rep = np.ascontiguousarray(np.broadcast_to(b, (P, DOUT)))

    shared = {"wTr": wTr, "G": G, "Gt": Gt, "eye": eye, "bias_rep": bias_rep}
    in_maps = []
    for c in range(N_CORES):
        xTc = np.ascontiguousarray(
            xf[c * TPC:(c + 1) * TPC].T.reshape(JT, P, TPC).transpose(1, 0, 2))
        in_maps.append({"xTr": xTc, **shared})

    trace = os.environ.get("KERNEL_TRACE", "0") == "1" and _maybe_enable_trace()
    res = run_bass_kernel_spmd(
        nc, in_maps, core_ids=list(range(N_CORES)), trace=trace,
        trace_cores=[0] if trace else None,
    )
    LAST_RESULT = res

    y = np.concatenate([res.results[c]["y"] for c in range(N_CORES)], axis=0)
    return np.ascontiguousarray(y.reshape(BATCH, SEQ, DOUT))


# revision 45
# speedup vs baseline: 1.4312x; 1.0184x over previous
"""Trainium2 Bass kernel for nn_GPTQOFTLinear.

y = (x rotated by block-diagonal Cayley(oft_r)) @ W^T + b

Strategy (8 NeuronCores, no collectives):
  - Data-parallel shard x over the 8192 tokens (1024 tokens/core); W, oft_r, b
    replicated.
  - On each core:
      1. Cayley transform packed as 32 block-diagonal 128x128 pairs:
         Q = F (I - C)^{-1} with S = skew(data), C = S@S, F = (I-S)^2
           = I - 2S + C.  The inverse is approximated by the 2-term Horner
         series (I - C)^{-1} ~= I + C + C^2 (|C|~0.05 so the truncation
         error ~|C|^3 ~ 1e-4).  3 matmuls/pair, fp16 operands, emitted in
         batched phases so the PE stream stays dense:
            psC = s2^T s2 = -4C          (s2 = 2S)
            psX = C @ (I + C)
            psQ = (C - 2S) @ X,   Q = X + psQ   (X = I + C + C^2)
      2. Rotate: x_rot^T[:, j] = Q_j^T-free matmuls (fp32r), result copied
         into a bf16 SBUF-resident x_rot^T [128, 32, 1024].
      3. Main matmul in bf16: y[t, o] = sum_j x_rot^T[j, t] * W^T[j, o] + b.
         W^T arrives per output-group as a 32 KiB/partition bf16 SBUF tile
         via a single gpsimd *casting* DMA (f32 HBM -> bf16 SBUF), so the
         inner loop has no DMA waits; 8 PSUM banks accumulate 8 token tiles.
  - Host side does only layout (shard/transpose/zero-pad/replicate), no math.
"""

import os
import sys

for _p in ("/opt/trn_rl_repo",):
    if _p not in sys.path and os.path.isdir(_p):
        sys.path.append(_p)

import numpy as np

import concourse.bass as bass  # noqa: E402
import concourse.mybir as mybir  # noqa: E402
import concourse.tile as tile  # noqa: E402
from concourse import bacc  # noqa: E402
from concourse.bass_utils import run_bass_kernel_spmd  # noqa: E402

# Problem shapes (hardcoded per contract).
BATCH, SEQ = 2, 4096
DIN = 4096
DOUT = 4096
BS = 64                      # oft block size
RANK = DIN // BS             # 64 blocks
N_CORES = 8
TOK = BATCH * SEQ            # 8192 tokens
TPC = TOK // N_CORES         # 1024 tokens per core
P = 128
JT = DIN // P                # 32 contraction tiles
NPAIR = RANK // 2            # 32 block pairs
NT = TPC // P                # 8 token tiles per core
OGW = 512                    # output-feature group width
OG = DOUT // OGW             # 8 output groups
XCH = 2                      # x^T j-tiles per staged DMA chunk
JH = JT // 2                 # j-tiles per W og-half tile
QGRP = 4                     # Cayley pair-group size
ALU = mybir.AluOpType

F32 = mybir.dt.float32
F32R = mybir.dt.float32r
F16 = mybir.dt.float16
BF16 = mybir.dt.bfloat16

# bf16: W tiles og-resident in SBUF via casting DMA, xrot bf16.
# f32r: baseline-style streamed f32r W tiles, xrot f32r.
_MODE = os.environ.get("KERNEL_MODE", "bf16")

_CACHE: dict = {}


def _emit_cayley(nc, tc, g_all, gt_all, eye_sb, Q, post_phase1=None):
    """Q[:, p, :] = Cayley(pair p), batched phases, fp16 operands.

    g_all/gt_all are the densely packed [P, NPAIR, BS] f32 tiles: partition
    quadrant 0:64 holds block 2p, 64:128 holds block 2p+1."""
    from contextlib import ExitStack

    with ExitStack() as ctx:
        arr = ctx.enter_context(tc.tile_pool(name="cayarr", bufs=1))
        cps = ctx.enter_context(tc.tile_pool(name="cps", bufs=4, space="PSUM"))
        xps = ctx.enter_context(tc.tile_pool(name="xps", bufs=2, space="PSUM"))
        qps = ctx.enter_context(tc.tile_pool(name="qps", bufs=2, space="PSUM"))

        def veng(i):
            return nc.vector if i % 2 == 0 else nc.gpsimd

        # Pair-groups of 4: separate tiles per group keep the dependency
        # tracker (whole-tile granularity) from serializing phases — group g
        # computes while group g+1's inputs land, and the first Q tiles are
        # ready early so the rotation can start sooner.
        GRP = QGRP
        NG = NPAIR // GRP

        # NOTE: gpsimd (Pool) cannot access PSUM on TRN2; PSUM-reading ops go
        # on DVE (tensor_tensor) or Activation (copy/scale only).
        for g in range(NG):
            s2 = arr.tile([P, GRP, P], F16, name=f"s2{g}", tag=f"s2{g}")
            C = arr.tile([P, GRP, P], F16, name=f"C{g}", tag=f"C{g}")
            X = arr.tile([P, GRP, P], F16, name=f"X{g}", tag=f"X{g}")

            # phase 1: s2 = g - gt (= 2S), block-diagonal; off-diagonal
            # quadrants zeroed (overlaps the g/gt DMA for later groups).
            nc.vector.memset(s2[:BS, :, BS:], 0)
            nc.gpsimd.memset(s2[BS:, :, :BS], 0)
            for i in range(GRP):
                p = g * GRP + i
                veng(p).tensor_sub(
                    s2[:BS, i, :BS], g_all[:BS, p, :], gt_all[:BS, p, :])
                veng(p + 1).tensor_sub(
                    s2[BS:, i, BS:], g_all[BS:, p, :], gt_all[BS:, p, :])
            if g == 0 and post_phase1 is not None:
                post_phase1()

            # phase 2: psC = s2^T @ s2 = -4C ; C = -0.25 psC (Act scale-copy)
            pcs = []
            for i in range(GRP):
                ps = cps.tile([P, P], F32, name="cps", tag="cps")
                nc.tensor.matmul(ps, s2[:, i, :], s2[:, i, :])
                pcs.append(ps)
            for i in range(GRP):
                # Split across Act and DVE so phase 3 isn't gated on one
                # engine's copy queue.
                if i % 2 == 0:
                    nc.scalar.mul(C[:, i, :], pcs[i], -0.25)
                else:
                    nc.vector.tensor_scalar_mul(C[:, i, :], pcs[i], -0.25)

            # phase 3: psX = C^T @ C = C^2 ; X = psX + C (DVE), X += I (gps)
            pxs = []
            for i in range(GRP):
                ps = xps.tile([P, P], F32, name="xps", tag="xps")
                nc.tensor.matmul(ps, C[:, i, :], C[:, i, :])
                pxs.append(ps)
            for i in range(GRP):
                nc.vector.tensor_add(X[:, i, :], pxs[i], C[:, i, :])
                nc.gpsimd.tensor_add(X[:, i, :], X[:, i, :], eye_sb)

            # phase 4: psQ = (C - 2S) @ X via two accumulating matmuls;
            #          Q = X + psQ = F @ X with F = I - 2S + C.
            for i in range(GRP):
                p = g * GRP + i
                ps = qps.tile([P, P], F32, name="qps", tag="qps")
                nc.tensor.matmul(ps, C[:, i, :], X[:, i, :],
                                 start=True, stop=False)
                nc.tensor.matmul(ps, s2[:, i, :], X[:, i, :],
                                 start=False, stop=True)
                nc.vector.tensor_add(Q[g][:, i, :], ps, X[:, i, :])


def _emit(nc, tc, xTr, wTr, G, Gt, eye, bias_rep, y):
    """Emit the whole per-core program under TileContext tc."""
    from contextlib import ExitStack

    xrot_dt = BF16 if _MODE == "bf16" else F32R

    ctx = ExitStack()
    with ctx:
        # ---- persistent pools (allocated first, stable addresses) ----
        const = ctx.enter_context(tc.tile_pool(name="const", bufs=1))
        qpool = ctx.enter_context(tc.tile_pool(name="qpool", bufs=1))
        xrot_pool = ctx.enter_context(tc.tile_pool(name="xrotp", bufs=1))
        if _MODE == "bf16":
            wt_pool = ctx.enter_context(tc.tile_pool(name="wtp", bufs=3))
        else:
            wt_pool = ctx.enter_context(tc.tile_pool(name="wtp", bufs=6))
        out_pool = ctx.enter_context(tc.tile_pool(name="outp", bufs=4))
        bias_pool = ctx.enter_context(tc.tile_pool(name="biasp", bufs=2))
        xstage_pool = ctx.enter_context(tc.tile_pool(name="xstagep", bufs=3))

        cayio_pool = ctx.enter_context(tc.tile_pool(name="cayio", bufs=1))

        # DMA priority order on the sync queue: eye + Cayley inputs first
        # (tiny, gate the whole pipeline), then x chunks, then per-og bias.
        eye_sb = const.tile([P, P], F32, name="eye_sb", tag="eye")
        nc.sync.dma_start(out=eye_sb, in_=eye)
        g_all = cayio_pool.tile([P, NPAIR, BS], F32, name="g_all", tag="g_all")
        nc.sync.dma_start(out=g_all, in_=G)
        gt_all = cayio_pool.tile([P, NPAIR, BS], F32, name="gt_all",
                                 tag="gt_all")
        nc.scalar.dma_start(out=gt_all, in_=Gt)

        Q = [qpool.tile([P, QGRP, P], F32R, name=f"Q{g}", tag=f"Q{g}")
             for g in range(NPAIR // QGRP)]
        xrot = xrot_pool.tile([P, JT, TPC], xrot_dt, name="xrot", tag="xrot")

        # x^T staged chunks, spread over THREE DMA initiator queues (sync,
        # Act HWDGE, gpsimd SWDGE) — transfers on one queue serialize, so a
        # single queue caps x at ~1/2 bandwidth.  Chunks 0-9 alternate
        # sync/Act up front; chunks 10+ go on gpsimd after the Cayley
        # phase-1 vector work (so their desc-gen doesn't delay it).
        NCH = JT // XCH
        xs_tiles = [
            xstage_pool.tile([P, XCH, TPC], F32R, name="xs", tag="xs")
            for _ in range(NCH)
        ]

        def issue_xs(c, eng):
            eng.dma_start(out=xs_tiles[c], in_=xTr[:, c * XCH:(c + 1) * XCH, :])

        for c in range(NCH - 2):
            issue_xs(c, nc.sync if c % 2 == 0 else nc.scalar)

        def gpsimd_xs():
            for c in range(NCH - 2, NCH):
                issue_xs(c, nc.gpsimd)

        # W og-half-tile prefetch (bf16 mode): gpsimd casting DMA f32 -> bf16
        # (casting DMAs are SWDGE-only).  Halves (16 j-tiles each) give finer
        # prefetch granularity at 3 bufs; the gpsimd queue carries only 2 x
        # chunks so og0's halves land before the main loop starts.
        wt_tiles = {}

        def issue_wt(og, half):
            if _MODE != "bf16" or og >= OG:
                return
            wt = wt_pool.tile([P, JH, OGW], BF16, name="wt", tag="wt")
            nc.gpsimd.dma_start(
                out=wt,
                in_=wTr[:, half * JH:(half + 1) * JH,
                        og * OGW:(og + 1) * OGW])
            wt_tiles[(og, half)] = wt

        # ---- Cayley (scoped; PSUM/SBUF freed before rotation/main) ----
        _emit_cayley(nc, tc, g_all, gt_all, eye_sb, Q, post_phase1=gpsimd_xs)

        # ---- rotation: x_rot^T[:, j, :] = Q_j^T.T @ x^T[j-tile] ----
        # (gpsimd cannot read PSUM: copies alternate Act / DVE)
        # W og0 prefetch starts late in the rotation so it doesn't steal DMA
        # bandwidth from the latency-critical x chunks.
        with tc.tile_pool(name="rpsum", bufs=4, space="PSUM") as rpsum:
            for j in range(JT):
                if j == 20:
                    issue_wt(0, 0)
                elif j == 30:
                    issue_wt(0, 1)
                xs = xs_tiles[j // XCH]
                for th in range(TPC // 512):
                    rps = rpsum.tile([P, 512], F32, name="rps", tag="rps")
                    nc.tensor.matmul(
                        rps,
                        Q[j // QGRP][:, j % QGRP, :],
                        xs[:, j % XCH, th * 512:(th + 1) * 512],
                    )
                    dst = xrot[:, j, th * 512:(th + 1) * 512]
                    if (2 * j + th) % 2 == 0:
                        nc.scalar.copy(out=dst, in_=rps)
                    else:
                        nc.vector.tensor_copy(out=dst, in_=rps)

        # ---- main matmul (all 8 PSUM banks) ----
        with tc.tile_pool(name="mpsum", bufs=1, space="PSUM") as mpsum:
            for og in range(OG):
                bias_og = bias_pool.tile([P, OGW], F32, name="bias_og",
                                         tag="bias_og")
                nc.sync.dma_start(
                    out=bias_og, in_=bias_rep[:, og * OGW:(og + 1) * OGW])

                # Two 4-bank passes per og (token tiles 0-3, then 4-7), on
                # alternating PSUM bank groups: pass k+1 never waits on pass
                # k's drains, so og boundaries cost nothing.
                for ps in range(2):
                    grp = "A" if ps == 0 else "B"
                    psums = [
                        mpsum.tile([P, OGW], F32, name=f"mp{grp}{t4}",
                                   tag=f"mp{grp}{t4}")
                        for t4 in range(4)
                    ]
                    if _MODE == "bf16":
                        for half in range(2):
                            if ps == 0:
                                issue_wt(og + 1, half)
                            wt = wt_tiles[(og, half)]
                            for jj in range(JH):
                                j = half * JH + jj
                                for t4 in range(4):
                                    tt = ps * 4 + t4
                                    nc.tensor.matmul(
                                        psums[t4],
                                        xrot[:, j, tt * P:(tt + 1) * P],
                                        wt[:, jj, :],
                                        start=(j == 0),
                                        stop=(j == JT - 1),
                                    )
                    else:
                        for j in range(JT):
                            wt = wt_pool.tile([P, OGW], F32R, name="wt",
                                              tag="wt")
                            nc.sync.dma_start(
                                out=wt,
                                in_=wTr[:, j, og * OGW:(og + 1) * OGW])
                            for t4 in range(4):
                                tt = ps * 4 + t4
                                nc.tensor.matmul(
                                    psums[t4],
                                    xrot[:, j, tt * P:(tt + 1) * P],
                                    wt[:],
                                    start=(j == 0),
                                    stop=(j == JT - 1),
                                )
                    # Drains: DVE adds bias directly from PSUM; odd tiles go
                    # Act copy (PSUM->SBUF) + gpsimd bias add (SBUF only).
                    for t4 in range(4):
                        tt = ps * 4 + t4
                        out_sb = out_pool.tile([P, OGW], F32, name="out_sb",
                                               tag="out_sb")
                        if t4 % 2 == 0:
                            nc.vector.tensor_add(out_sb, psums[t4], bias_og)
                        else:
                            nc.scalar.copy(out=out_sb, in_=psums[t4])
                            nc.gpsimd.tensor_add(out_sb, out_sb, bias_og)
                        yeng = nc.sync if t4 % 2 == 0 else nc.scalar
                        yeng.dma_start(
                            out=y[tt * P:(tt + 1) * P,
                                  og * OGW:(og + 1) * OGW],
                            in_=out_sb)


def _build():
    key = _MODE
    if key in _CACHE:
        return _CACHE[key]
    nc = bacc.Bacc("TRN2", target_bir_lowering=False, debug=False,
                   num_devices=N_CORES)
    xTr = nc.dram_tensor("xTr", [P, JT, TPC], F32R, kind="ExternalInput").ap()
    wTr = nc.dram_tensor("wTr", [P, JT, DOUT], F32, kind="ExternalInput").ap()
    G = nc.dram_tensor("G", [P, NPAIR, BS], F32, kind="ExternalInput").ap()
    Gt = nc.dram_tensor("Gt", [P, NPAIR, BS], F32, kind="ExternalInput").ap()
    eye = nc.dram_tensor("eye", [P, P], F32, kind="ExternalInput").ap()
    bias_rep = nc.dram_tensor("bias_rep", [P, DOUT], F32,
                              kind="ExternalInput").ap()
    y = nc.dram_tensor("y", [TPC, DOUT], F32, kind="ExternalOutput").ap()

    with tile.TileContext(nc) as tc:
        _emit(nc, tc, xTr, wTr, G, Gt, eye, bias_rep, y)
    nc.compile()
    _CACHE[key] = nc
    return nc


def _maybe_enable_trace():
    """Inject the NTFF profile hook so run_bass_kernel_spmd(trace=True) works
    under axon in this container.  Only used by the dev harness."""
    import types
    try:
        import antenv
        from trn_agent_boot.trn_boot import _ntff_profile_via_ctypes
        import concourse.bass_utils as bass_utils
        hook = _ntff_profile_via_ctypes("/opt/axon/libaxon_pjrt.so")
        mod = types.ModuleType("antenv.axon_hooks")
        mod.get_axon_ntff_profile_hook = lambda: hook
        mod.set_axon_ntff_profile_hook = lambda h: None
        sys.modules["antenv.axon_hooks"] = mod
        antenv.axon_hooks = mod
        bass_utils.upload_artifacts = lambda tmpdir: "local://" + tmpdir
        return True
    except Exception:
        return False


LAST_RESULT = None


def kernel(x, oft_r, W, b):
    global LAST_RESULT
    x = np.ascontiguousarray(np.asarray(x, dtype=np.float32))
    oft_r = np.asarray(oft_r, dtype=np.float32)
    W = np.asarray(W, dtype=np.float32)
    b = np.asarray(b, dtype=np.float32)

    nc = _build()

    # Host-side layout only (no arithmetic): shard/transpose/pad/replicate.
    xf = x.reshape(TOK, DIN)
    wTr = np.ascontiguousarray(
        W.T.reshape(JT, P, DOUT).transpose(1, 0, 2))
    # Dense block packing: partitions 0:64 hold block 2p, 64:128 block 2p+1.
    G = np.zeros((P, NPAIR, BS), np.float32)
    Gt = np.zeros((P, NPAIR, BS), np.float32)
    oft_t = oft_r.transpose(0, 2, 1)
    for p in range(NPAIR):
        G[:BS, p, :] = oft_r[2 * p]
        G[BS:, p, :] = oft_r[2 * p + 1]
        Gt[:BS, p, :] = oft_t[2 * p]
        Gt[BS:, p, :] = oft_t[2 * p + 1]
    eye = np.eye(P, dtype=np.float32)
    bias_rep = np.ascontiguousarray(np.broadcast_to(b, (P, DOUT)))

    shared = {"wTr": wTr, "G": G, "Gt": Gt, "eye": eye, "bias_rep": bias_rep}
    in_maps = []
    for c in range(N_CORES):
        xTc = np.ascontiguousarray(
            xf[c * TPC:(c + 1) * TPC].T.reshape(JT, P, TPC).transpose(1, 0, 2))
        in_maps.append({"xTr": xTc, **shared})

    trace = os.environ.get("KERNEL_TRACE", "0") == "1" and _maybe_enable_trace()
    res = run_bass_kernel_spmd(
        nc, in_maps, core_ids=list(range(N_CORES)), trace=trace,
        trace_cores=[0] if trace else None,
    )
    LAST_RESULT = res

    y = np.concatenate([res.results[c]["y"] for c in range(N_CORES)], axis=0)
    return np.ascontiguousarray(y.reshape(BATCH, SEQ, DOUT))
